# revision 47
# baseline (speedup 1.0000x reference)
"""Trainium2 Bass kernel: single-head causal attention (B=4, T=4096, C=2048, H=128).

    q = x @ Wq; k = x @ Wk; v = x @ Wv        (per batch element)
    out = softmax(causal(q k^T * C**-0.5)) @ v

Sharding: two cores per batch element (8 cores, B=4). Within a batch the
4096 q rows are split between the pair by 128-row-block parity (core p owns
blocks p, p+2, ...), which balances the causal work. There are NO
collectives: each core projects k/v for BOTH parity halves (its own xT plus
the partner's xO slab are both inputs), trading ~27us of redundant PE work
for the two ~41us AllGathers of the previous design.

Pipeline: x arrives in eight 512-column groups (own/other alternating per
superblock). Each group is projected on arrival (kT/qT with W stationary;
V directly in [t,h] layout with the x-chunk stationary, packing 4 t-blocks
into one PSUM bank via the has_written accumulate rule). After group pair s
lands, causal attention "wave" s runs: for each 128-wide k block,
S^T[k,q] = kT_blk^T @ qT_s on the tensor engine, P = exp(S^T * scale) on
the scalar engine (|scaled scores| < ~2 so un-maxed exp is safe), causal
masking multiplies 0/1 mask data on diagonal-band chunks only (per-core
mask data keeps the instruction stream SPMD-uniform), and O^T[h,q]
accumulates V_blk^T... i.e. matmul(V_blk stationary, P moving) in a single
PSUM bank across the whole wave. Softmax denominators are partition-axis
sums of P done on the otherwise-idle GpSimd engine; the finalize divides
O^T by den, transposes back to [q,h] via PE-transpose, and streams out.
Projection groups are interleaved between waves in issue order so the
tensor engine never waits on the scalar engine's exp.
"""

import numpy as np
import ml_dtypes

B, T, C, H = 4, 4096, 2048, 128
NCORES = 8
TQ = T // 2              # per-core q rows (parity half)
NCC = C // 128           # 16 contraction chunks
NSB = TQ // 512          # 4 q superblocks of 512 rows per core
NG = 4                   # x column groups of 512 per parity slab
SCALE = float(C) ** -0.5
BF16 = ml_dtypes.bfloat16

# mask slots for the diagonal band, applied to S^T-exp chunks [128k x 128q].
# own-parity k block l vs q chunk c of superblock s: dd = l - 4s; a multiply
# is needed only when dd >= c (tri when dd == c, zero when dd > c; parity-
# independent). other-parity k block: mul when dd >= c with per-core data
# (p=0: zero; p=1: ones when dd == c else zero).
DIAG = [(dd, c) for dd in range(4) for c in range(4) if dd >= c]
SLOT = {("own", s): i for i, s in enumerate(DIAG)}
SLOT.update({("oth", s): len(DIAG) + i for i, s in enumerate(DIAG)})
NSLOT = 2 * len(DIAG)  # 20

_cached = {}


def _build_nc():
    import concourse.bacc as bacc
    import concourse.mybir as mybir
    from concourse import tile

    f32 = mybir.dt.float32
    bf16 = mybir.dt.bfloat16
    AF = mybir.ActivationFunctionType
    AX = mybir.AxisListType
    ALU = mybir.AluOpType

    nc = bacc.Bacc("TRN2", target_bir_lowering=False, debug=False,
                   num_devices=NCORES)

    xT = nc.declare_dram_parameter("xT", [C, TQ], bf16, isOutput=False)
    xO = nc.declare_dram_parameter("xO", [C, TQ], bf16, isOutput=False)
    # weights arrive host-transposed to the SBUF layout [128, cc*128] so the
    # DMA moves 4KB contiguous runs per partition (256B runs pay a 2x penalty)
    wq = nc.declare_dram_parameter("Wq", [128, NCC * H], bf16, isOutput=False)
    wk = nc.declare_dram_parameter("Wk", [128, NCC * H], bf16, isOutput=False)
    wv = nc.declare_dram_parameter("Wv", [128, NCC * H], bf16, isOutput=False)
    msk = nc.declare_dram_parameter("masks", [128, NSLOT * 128], bf16,
                                    isOutput=False)
    idn = nc.declare_dram_parameter("ident32", [128, 128], f32,
                                    isOutput=False)
    out = nc.declare_dram_parameter("out", [TQ, H], f32, isOutput=True)

    with tile.TileContext(nc) as tc:
        with tc.tile_pool(name="sb", bufs=1) as sb, \
             tc.tile_pool(name="xp", bufs=3) as xp, \
             tc.tile_pool(name="pp", bufs=8) as pp, \
             tc.tile_pool(name="dp", bufs=2) as dp, \
             tc.tile_pool(name="fin", bufs=2) as fin, \
             tc.tile_pool(name="ps_k", bufs=1, space="PSUM") as pk, \
             tc.tile_pool(name="ps_q", bufs=1, space="PSUM") as pq, \
             tc.tile_pool(name="ps_v", bufs=1, space="PSUM") as pv, \
             tc.tile_pool(name="ps_s", bufs=3, space="PSUM") as ps, \
             tc.tile_pool(name="ps_o", bufs=2, space="PSUM") as po:

            # ---- small resident loads ------------------------------
            w_sb = {}
            w_dram = {"wk": wk, "wq": wq, "wv": wv}
            for name in ("wk", "wq", "wv"):
                t = sb.tile([128, NCC * H], bf16, tag=name)
                w_sb[name] = t

            def w_dma(name):
                nc.sync.dma_start(w_sb[name][:], w_dram[name].ap())

            mask_sb = sb.tile([128, NSLOT * 128], bf16)
            ident = sb.tile([128, 128], f32)
            ones1 = sb.tile([128, 1], bf16)
            nc.gpsimd.memset(ones1[:], 1.0)

            def wchunk(name, cc):
                return w_sb[name][:, cc * H:(cc + 1) * H]

            kT = {"own": sb.tile([128, TQ], bf16, tag="ktw", name="kT_own"),
                  "oth": sb.tile([128, TQ], bf16, tag="kto", name="kT_oth")}
            V = {"own": sb.tile([128, TQ], bf16, tag="vw", name="V_own"),
                 "oth": sb.tile([128, TQ], bf16, tag="vo", name="V_oth")}
            qT = sb.tile([128, TQ], bf16)

            # ---- projection of one 512-col x group, split into
            # schedulable pieces: dma / per-cc matmuls / copies --------
            def proj_alloc(par, g):
                # four sub-tiles per group so each projection matmul depends
                # only on the sub-DMA that carries its cc chunks
                subs = [xp.tile([128, 4 * 512], bf16, tag="xg",
                                name=f"xg_{par}{g}_{d}") for d in range(4)]
                own = par == "own"
                kps = pk.tile([128, 512], f32, tag="kps", name=f"kps{g}")
                vps = pv.tile([128, 512], f32, tag="vps", name=f"vps{g}")
                qps = (pq.tile([128, 512], f32, tag="qps", name=f"qps{g}")
                       if own else None)
                return (par, g, subs, kps, vps, qps)

            def proj_sub_dma(st8, d, src):
                par, g, subs, kps, vps, qps = st8
                nc.sync.dma_start(
                    subs[d][:].rearrange("p (n t) -> p n t", t=512),
                    src.ap()[:, 512 * g:512 * (g + 1)]
                    .rearrange("(n p) t -> p n t", p=128)[:, 4 * d:
                                                          4 * d + 4, :])

            def proj_dma(par, g, src):
                st8 = proj_alloc(par, g)
                for d in range(4):
                    proj_sub_dma(st8, d, src)
                return st8

            def proj_cc(st8, cc):
                par, g, subs, kps, vps, qps = st8
                st, sp = cc == 0, cc == NCC - 1
                xcc = subs[cc // 4][:].rearrange(
                    "p (n t) -> p n t", t=512)[:, cc % 4, :]
                nc.tensor.matmul(kps[:], wchunk("wk", cc), xcc,
                                 start=st, stop=sp)
                if qps is not None:
                    nc.tensor.matmul(qps[:], wchunk("wq", cc),
                                     xcc, start=st, stop=sp)
                # V in [t, h] layout: x chunk stationary, Wv moving;
                # 4 t-blocks share one PSUM bank (start only clears once)
                for b in range(4):
                    nc.tensor.matmul(
                        vps[:, 128 * b:128 * (b + 1)],
                        xcc[:, 128 * b:128 * (b + 1)],
                        wchunk("wv", cc),
                        start=(st and b == 0), stop=(sp and b == 3),
                        skip_group_check=True)

            def proj_copies(st8):
                # emitted at a point where the group's matmuls have already
                # executed, so these never block their engine's queue head
                par, g, subs, kps, vps, qps = st8
                cols = slice(512 * g, 512 * (g + 1))
                nc.scalar.copy(kT[par][:, cols], kps[:])
                if qps is not None:
                    nc.scalar.copy(qT[:, cols], qps[:])
                nc.vector.tensor_copy(V[par][:, cols], vps[:])

            # ---- one attention chunk (wave s, k block l, parity).
            # Diagonal-band chunks (dd >= 0) only compute the causally
            # valid q columns [128*dd, 512): the columns below are zero
            # after masking anyway, so skip their scores/exp/AV/den
            # entirely; only the c == dd boundary block needs a mask.
            def chunk(s, l, par, ot, wden, first_pool, dps=None,
                      dfirst=False, dlast=False):
                first = par == "own" and l == 0
                last = par == "oth" and l == 4 * s + 3
                dd = l - 4 * s
                off = 128 * dd if dd > 0 else 0
                width = 512 - off
                sps = ps.tile([128, 512], f32, tag="s", name=f"s{s}_{l}{par}")
                nc.tensor.matmul(sps[:, 0:width],
                                 kT[par][:, 128 * l:128 * (l + 1)],
                                 qT[:, 512 * s + off:512 * (s + 1)],
                                 start=True, stop=True)
                P = pp.tile([128, 512], bf16, tag="p", name=f"p{s}_{l}{par}")
                nc.scalar.activation(P[:, 0:width], sps[:, 0:width],
                                     AF.Exp, scale=SCALE)
                if dd >= 0:
                    si = SLOT[(par, (dd, dd))]
                    nc.vector.tensor_mul(
                        P[:, 0:128], P[:, 0:128],
                        mask_sb[:, 128 * si:128 * (si + 1)])
                nc.tensor.matmul(ot[:, off:512],
                                 V[par][:, 128 * l:128 * (l + 1)],
                                 P[:, 0:width], start=first, stop=last,
                                 skip_group_check=True)
                if dps is None:
                    # denominator contribution: partition-sum of P on the
                    # idle GpSimd engine (to partition 0 of a scratch; SBUF
                    # partition offsets must be quadrant-aligned). The DVE
                    # running add into the wave accumulator is emitted a few
                    # chunks later (returned to the caller) so a Pool
                    # backlog can never block the DVE queue head.
                    if first_pool:
                        nc.gpsimd.tensor_reduce(wden[0:1, :], P[:, 0:width],
                                                axis=AX.C, op=ALU.add)
                    else:
                        dsc = dp.tile([1, 512], f32, tag="dpart",
                                      name=f"dsc{s}_{l}{par}", bufs=10)
                        nc.gpsimd.tensor_reduce(dsc[0:1, 0:width],
                                                P[:, 0:width],
                                                axis=AX.C, op=ALU.add)
                        return (dsc, off, width)
                else:
                    # Pool saturates late in a wave: late chunks put the
                    # denominator on the PE via a ones-column matmul
                    nc.tensor.matmul(dps[0:1, off:512], ones1[:],
                                     P[:, 0:width],
                                     start=dfirst, stop=dlast,
                                     skip_group_check=True)
                return None

            def finalize(s, ot, wden, dps=None):
                if dps is not None:
                    nc.vector.tensor_add(wden[0:1, :], wden[0:1, :],
                                         dps[0:1, :])
                rec = fin.tile([1, 512], f32, tag="rec", name=f"rec{s}")
                nc.vector.reciprocal_approx_fast(rec[:], wden[0:1, :])
                recB = fin.tile([128, 512], f32, tag="recB", name=f"recB{s}")
                nc.gpsimd.partition_broadcast(recB[:], rec[:])
                otn = fin.tile([128, 512], f32, tag="otn", name=f"otn{s}")
                nc.vector.tensor_mul(otn[:], ot[:], recB[:])
                otr = ps.tile([128, 512], f32, tag="s", name=f"otr{s}")
                for c in range(4):
                    nc.tensor.matmul(otr[:, 128 * c:128 * (c + 1)],
                                     otn[:, 128 * c:128 * (c + 1)], ident[:],
                                     is_transpose=True,
                                     start=(c == 0), stop=(c == 3),
                                     skip_group_check=True)
                osb = fin.tile([128, 512], f32, tag="osb", name=f"osb{s}")
                nc.scalar.copy(osb[:], otr[:])
                nc.sync.dma_start(
                    out.ap()[512 * s:512 * (s + 1), :]
                    .rearrange("(n p) h -> p n h", p=128),
                    osb[:].rearrange("p (n h) -> p n h", h=H))

            # ---- merged schedule: proj groups chased by waves.
            # Wave s's chunk stream has group-(s+1) projection cc-iters
            # spread over its first ~60% so the PE always has matmul work
            # while ACT runs exp; the group's psum->SBUF copies land at the
            # wave tail where their waits are already satisfied. The
            # other-parity group 3 is deferred into wave 3 (its k blocks
            # l>=12 come last) to give wave 3 the same PE filler. --------
            st_own0 = proj_alloc("own", 0)
            w_dma("wk")
            proj_sub_dma(st_own0, 0, xT)
            w_dma("wq")
            proj_sub_dma(st_own0, 1, xT)
            w_dma("wv")
            proj_sub_dma(st_own0, 2, xT)
            proj_sub_dma(st_own0, 3, xT)
            for cc in range(NCC):
                proj_cc(st_own0, cc)
            nc.sync.dma_start(mask_sb[:], msk.ap())
            nc.sync.dma_start(ident[:], idn.ap())
            st_oth0 = proj_dma("oth", 0, xO)
            for cc in range(NCC):
                proj_cc(st_oth0, cc)
            proj_copies(st_own0)
            proj_copies(st_oth0)

            pend_w3 = []
            pending_fin = None
            for s in range(NSB):
                if s == NSB - 2:
                    # wave 2 carries only own-g3; oth-g3 goes into wave 3
                    pend = [proj_dma("own", s + 1, xT)]
                    pend_w3 = [proj_dma("oth", s + 1, xO)]
                elif s == NSB - 1:
                    pend = pend_w3
                else:
                    pend = [proj_dma("own", s + 1, xT),
                            proj_dma("oth", s + 1, xO)]
                chunks = [(l, par) for l in range(4 * s + 4)
                          for par in ("own", "oth")]
                # spread the pending proj cc-iters over the early chunks,
                # leaving the wave tail free so the copies' waits are met
                spread_over = 20 if s == NSB - 1 else \
                    max(1, (len(chunks) * 6) // 10)
                cciters = [(st8, cc) for st8 in pend for cc in range(NCC)]
                ncci = len(cciters)
                ot = po.tile([128, 512], f32, tag="ot", name=f"ot{s}")
                # Pool saturates late in long waves, so tail chunks (with
                # the q psum bank free by then) switch to a PE ones-matmul
                # denominator accumulated in that bank
                pe_den_from = {0: 8, 1: 16, 2: 24, 3: 20}[s]
                dps = (pq.tile([128, 512], f32, tag="qps", name="dps")
                       if pe_den_from < len(chunks) else None)
                wden = dp.tile([1, 512], f32, tag="wden", name=f"wden{s}",
                               bufs=2)
                done_copies = not cciters
                pend_adds = []

                def flush_adds(upto):
                    while pend_adds and pend_adds[0][0] <= upto:
                        _, (dsc, aoff, awid) = pend_adds.pop(0)
                        nc.vector.tensor_add(wden[0:1, aoff:aoff + awid],
                                             wden[0:1, aoff:aoff + awid],
                                             dsc[0:1, 0:awid])

                for i, (l, par) in enumerate(chunks):
                    if i < pe_den_from:
                        add = chunk(s, l, par, ot, wden, i == 0)
                        if add is not None:
                            pend_adds.append((i, add))
                    else:
                        chunk(s, l, par, ot, wden, False, dps=dps,
                              dfirst=(i == pe_den_from),
                              dlast=(i == len(chunks) - 1))
                    flush_adds(i - 6)
                    if i == 6 and pending_fin is not None:
                        # previous wave's finalize chain overlaps this
                        # wave's early chunks instead of stalling the PE
                        finalize(*pending_fin)
                        pending_fin = None
                    if cciters and i < spread_over:
                        hi = ncci * (i + 1) // spread_over
                        while len(cciters) > ncci - hi:
                            proj_cc(*cciters.pop(0))
                    if not cciters and not done_copies:
                        for st8 in pend:
                            proj_copies(st8)
                        done_copies = True
                if not done_copies:
                    for st8 in pend:
                        proj_copies(st8)
                flush_adds(len(chunks))
                pending_fin = (s, ot, wden, dps)
            finalize(*pending_fin)

    nc.finalize()
    return nc


def _build_masks(p):
    kk = np.arange(128)[:, None]   # k index (partition of S^T)
    tt = np.arange(128)[None, :]   # q index
    tri = (kk <= tt).astype(np.float32)
    ones = np.ones((128, 128), np.float32)
    zero = np.zeros((128, 128), np.float32)
    M = np.zeros((128, NSLOT * 128), np.float32)
    for (dd, c) in DIAG:
        M[:, SLOT[("own", (dd, c))] * 128:][:, :128] = \
            tri if dd == c else zero
        if p == 0:
            m = zero
        else:
            m = ones if dd == c else zero
        M[:, SLOT[("oth", (dd, c))] * 128:][:, :128] = m
    return np.ascontiguousarray(M.astype(BF16))


def _get_nc():
    if "nc" not in _cached:
        _cached["nc"] = _build_nc()
        _cached["masks"] = {p: _build_masks(p) for p in (0, 1)}
        _cached["ident32"] = np.ascontiguousarray(np.eye(128, dtype=np.float32))
    return _cached["nc"]


def _prep_in_maps(x, Wq, Wk, Wv):
    _get_nc()
    w16 = {}
    for n, w in (("Wq", Wq), ("Wk", Wk), ("Wv", Wv)):
        # SBUF layout [p, cc*128+h] = W[cc*128+p, h]
        wt = np.asarray(w).astype(BF16).reshape(NCC, 128, H)
        w16[n] = np.ascontiguousarray(
            wt.transpose(1, 0, 2).reshape(128, NCC * H))
    xTs = {}
    for b in range(B):
        xb = np.asarray(x[b])
        for p in range(2):
            sl = xb.reshape(T // 128, 128, C)[p::2].reshape(TQ, C)
            xTs[(b, p)] = np.ascontiguousarray(sl.astype(BF16).T)
    in_maps = []
    for c in range(NCORES):
        b, p = divmod(c, 2)
        in_maps.append({"xT": xTs[(b, p)], "xO": xTs[(b, 1 - p)],
                        "masks": _cached["masks"][p],
                        "ident32": _cached["ident32"], **w16})
    return in_maps


def _gather_out(results):
    out = np.empty((B, T, H), np.float32)
    for c in range(NCORES):
        b, p = divmod(c, 2)
        out[b].reshape(T // 128, 128, H)[p::2] = \
            results[c]["out"].reshape(TQ // 128, 128, H)
    return out


def kernel(x, Wq, Wk, Wv):
    from concourse.bass_utils import run_bass_kernel_spmd

    nc = _get_nc()
    in_maps = _prep_in_maps(x, Wq, Wk, Wv)
    res = run_bass_kernel_spmd(nc, in_maps, list(range(NCORES)))
    return _gather_out(res.results)


# revision 63
# speedup vs baseline: 1.0009x; 1.0009x over previous
"""Trainium2 Bass kernel: single-head causal attention (B=4, T=4096, C=2048, H=128).

    q = x @ Wq; k = x @ Wk; v = x @ Wv        (per batch element)
    out = softmax(causal(q k^T * C**-0.5)) @ v

Sharding: two cores per batch element (8 cores, B=4). Within a batch the
4096 q rows are split between the pair by 128-row-block parity (core p owns
blocks p, p+2, ...), which balances the causal work. There are NO
collectives: each core projects k/v for BOTH parity halves (its own xT plus
the partner's xO slab are both inputs), trading ~27us of redundant PE work
for the two ~41us AllGathers of the previous design.

Pipeline: x arrives in eight 512-column groups, DMA'd one wave ahead of
use in four sub-tiles each. Each group is projected as it lands (q first,
then k with W stationary; V directly in [t,h] layout with the x-chunk
stationary, packing 4 t-blocks into one PSUM bank via the has_written
accumulate rule). q/k are cast to fp8e4 and repacked by an SBUF-to-SBUF
DMA into the [64, 2, t] DoubleRow layout, so score matmuls run at 0.5
cycles/row. Causal attention "wave" s then runs: per 128-wide k block,
S^T[k,q] = sum_i kT8[:,i,blk].T @ qT8[:,i,cols] (DoubleRow), P =
exp(S^T * scale) on the scalar engine (|scaled scores| < ~2 so un-maxed
exp is safe), a single 0/1 mask multiply on the c == dd boundary block
(diagonal chunks only compute their causally valid q columns; per-core
mask data keeps the instruction stream SPMD-uniform), and O^T[h,q] +=
matmul(V_blk stationary, P moving) accumulates in one PSUM bank across
the wave. Softmax denominators are partition-sums of P on the otherwise
idle GpSimd engine (DVE running-add, emitted a few chunks late to dodge
queue head-of-line blocking), switching to a PE ones-matmul for late
chunks where Pool would saturate. The finalize (deferred into the next
wave) divides O^T by den, PE-transposes back to [q,h], and streams out.
Projection matmuls of group s+1 are spread between wave-s chunks in issue
order so the tensor engine never waits on the scalar engine's exp.
"""

import numpy as np
import ml_dtypes

B, T, C, H = 4, 4096, 2048, 128
NCORES = 8
TQ = T // 2              # per-core q rows (parity half)
NCC = C // 128           # 16 contraction chunks
NSB = TQ // 512          # 4 q superblocks of 512 rows per core
NG = 4                   # x column groups of 512 per parity slab
SCALE = float(C) ** -0.5
BF16 = ml_dtypes.bfloat16

# mask slots for the diagonal band, applied to S^T-exp chunks [128k x 128q].
# own-parity k block l vs q chunk c of superblock s: dd = l - 4s; a multiply
# is needed only when dd >= c (tri when dd == c, zero when dd > c; parity-
# independent). other-parity k block: mul when dd >= c with per-core data
# (p=0: zero; p=1: ones when dd == c else zero).
DIAG = [(dd, c) for dd in range(4) for c in range(4) if dd >= c]
SLOT = {("own", s): i for i, s in enumerate(DIAG)}
SLOT.update({("oth", s): len(DIAG) + i for i, s in enumerate(DIAG)})
NSLOT = 2 * len(DIAG)  # 20

_cached = {}


def _build_nc():
    import concourse.bacc as bacc
    import concourse.mybir as mybir
    from concourse import tile

    f32 = mybir.dt.float32
    bf16 = mybir.dt.bfloat16
    f8 = mybir.dt.float8e4
    AF = mybir.ActivationFunctionType
    AX = mybir.AxisListType
    ALU = mybir.AluOpType
    DR = mybir.MatmulPerfMode.DoubleRow

    nc = bacc.Bacc("TRN2", target_bir_lowering=False, debug=False,
                   num_devices=NCORES)

    xT = nc.declare_dram_parameter("xT", [C, TQ], bf16, isOutput=False)
    xO = nc.declare_dram_parameter("xO", [C, TQ], bf16, isOutput=False)
    # weights arrive host-transposed to the SBUF layout [128, cc*128] so the
    # DMA moves 4KB contiguous runs per partition (256B runs pay a 2x penalty)
    wq = nc.declare_dram_parameter("Wq", [128, NCC * H], bf16, isOutput=False)
    wk = nc.declare_dram_parameter("Wk", [128, NCC * H], bf16, isOutput=False)
    wv = nc.declare_dram_parameter("Wv", [128, NCC * H], bf16, isOutput=False)
    msk = nc.declare_dram_parameter("masks", [128, NSLOT * 128], bf16,
                                    isOutput=False)
    idn = nc.declare_dram_parameter("ident32", [128, 128], f32,
                                    isOutput=False)
    out = nc.declare_dram_parameter("out", [TQ, H], f32, isOutput=True)

    with tile.TileContext(nc) as tc:
        with tc.tile_pool(name="sb", bufs=1) as sb, \
             tc.tile_pool(name="xp", bufs=3) as xp, \
             tc.tile_pool(name="pp", bufs=8) as pp, \
             tc.tile_pool(name="dp", bufs=2) as dp, \
             tc.tile_pool(name="fin", bufs=3) as fin, \
             tc.tile_pool(name="ps_k", bufs=1, space="PSUM") as pk, \
             tc.tile_pool(name="ps_q", bufs=1, space="PSUM") as pq, \
             tc.tile_pool(name="ps_v", bufs=1, space="PSUM") as pv, \
             tc.tile_pool(name="ps_s", bufs=3, space="PSUM") as ps, \
             tc.tile_pool(name="ps_o", bufs=2, space="PSUM") as po:

            # ---- small resident loads ------------------------------
            w_sb = {}
            w_dram = {"wk": wk, "wq": wq, "wv": wv}
            for name in ("wk", "wq", "wv"):
                t = sb.tile([128, NCC * H], bf16, tag=name)
                w_sb[name] = t

            def w_dma(name):
                nc.sync.dma_start(w_sb[name][:], w_dram[name].ap())

            mask_sb = sb.tile([128, NSLOT * 128], bf16)
            ident = sb.tile([128, 128], f32)
            ones1 = sb.tile([128, 1], bf16)
            nc.gpsimd.memset(ones1[:], 1.0)

            def wchunk(name, cc):
                return w_sb[name][:, cc * H:(cc + 1) * H]

            # q/k live in fp8 packed for DoubleRow matmuls: [64, 2, t] with
            # slab[p, i, t] = proj[64*i + p, t] (contraction pairs per
            # partition). Scores S = sum_i kT8[:,i,blk].T @ qT8[:,i,cols].
            kT8 = {"own": sb.tile([64, 2 * TQ], f8, tag="ktw", name="kT_own"),
                   "oth": sb.tile([64, 2 * TQ], f8, tag="kto", name="kT_oth")}
            V = {"own": sb.tile([128, TQ], bf16, tag="vw", name="V_own"),
                 "oth": sb.tile([128, TQ], bf16, tag="vo", name="V_oth")}
            qT8 = sb.tile([64, 2 * TQ], f8)

            def slab3(t):
                return t[:].rearrange("p (i t) -> p i t", i=2)

            # ---- projection of one 512-col x group, split into
            # schedulable pieces: dma / per-cc matmuls / copies --------
            def proj_alloc(par, g):
                # four sub-tiles per group so each projection matmul depends
                # only on the sub-DMA that carries its cc chunks
                subs = [xp.tile([128, 4 * 512], bf16, tag="xg",
                                name=f"xg_{par}{g}_{d}") for d in range(4)]
                own = par == "own"
                kps = pk.tile([128, 512], f32, tag="kps", name=f"kps{g}")
                vps = pv.tile([128, 512], f32, tag="vps", name=f"vps{g}")
                qps = (pq.tile([128, 512], f32, tag="qps", name=f"qps{g}")
                       if own else None)
                return (par, g, subs, kps, vps, qps)

            def proj_sub_dma(st8, d, src):
                par, g, subs, kps, vps, qps = st8
                nc.sync.dma_start(
                    subs[d][:].rearrange("p (n t) -> p n t", t=512),
                    src.ap()[:, 512 * g:512 * (g + 1)]
                    .rearrange("(n p) t -> p n t", p=128)[:, 4 * d:
                                                          4 * d + 4, :])

            def proj_dma(par, g, src):
                st8 = proj_alloc(par, g)
                for d in range(4):
                    proj_sub_dma(st8, d, src)
                return st8

            def _xcc(subs, cc):
                return subs[cc // 4][:].rearrange(
                    "p (n t) -> p n t", t=512)[:, cc % 4, :]

            def proj_cc_q(st8, cc):
                par, g, subs, kps, vps, qps = st8
                nc.tensor.matmul(qps[:], wchunk("wq", cc), _xcc(subs, cc),
                                 start=cc == 0, stop=cc == NCC - 1)

            def proj_cc(st8, cc):
                par, g, subs, kps, vps, qps = st8
                st, sp = cc == 0, cc == NCC - 1
                xcc = _xcc(subs, cc)
                nc.tensor.matmul(kps[:], wchunk("wk", cc), xcc,
                                 start=st, stop=sp)
                if qps is not None and g == 0:
                    nc.tensor.matmul(qps[:], wchunk("wq", cc),
                                     xcc, start=st, stop=sp)
                # V in [t, h] layout: x chunk stationary, Wv moving;
                # 4 t-blocks share one PSUM bank (start only clears once)
                for b in range(4):
                    nc.tensor.matmul(
                        vps[:, 128 * b:128 * (b + 1)],
                        xcc[:, 128 * b:128 * (b + 1)],
                        wchunk("wv", cc),
                        start=(st and b == 0), stop=(sp and b == 3),
                        skip_group_check=True)

            def to_fp8_slab(slab, g, psum, nm):
                # cast psum -> fp8 stage, then an SBUF->SBUF DMA folds
                # partitions 64..127 into the [64, 2, t] DoubleRow layout
                stage = xp.tile([128, 512], f8, tag="st8", name=f"st8_{nm}",
                                bufs=3)
                nc.scalar.copy(stage[:], psum[:])
                for i in range(2):
                    nc.sync.dma_start(
                        slab3(slab)[:, i, 512 * g:512 * (g + 1)],
                        stage[64 * i:64 * (i + 1), :])

            def proj_copy_q(st8):
                par, g, subs, kps, vps, qps = st8
                to_fp8_slab(qT8, g, qps, f"q{g}")

            def proj_copies(st8):
                # emitted at a point where the group's matmuls have already
                # executed, so these never block their engine's queue head
                par, g, subs, kps, vps, qps = st8
                cols = slice(512 * g, 512 * (g + 1))
                to_fp8_slab(kT8[par], g, kps, f"k{par}{g}")
                if qps is not None and g == 0:
                    proj_copy_q(st8)
                nc.vector.tensor_copy(V[par][:, cols], vps[:])

            # ---- one attention chunk (wave s, k block l, parity).
            # Diagonal-band chunks (dd >= 0) only compute the causally
            # valid q columns [128*dd, 512): the columns below are zero
            # after masking anyway, so skip their scores/exp/AV/den
            # entirely; only the c == dd boundary block needs a mask.
            def chunk(s, l, par, ot, wden, first_pool, dps=None,
                      dfirst=False, dlast=False):
                first = par == "own" and l == 0
                last = par == "oth" and l == 4 * s + 3
                dd = l - 4 * s
                off = 128 * dd if dd > 0 else 0
                width = 512 - off
                sps = ps.tile([128, 512], f32, tag="s", name=f"s{s}_{l}{par}")
                nc.tensor.matmul(sps[:, 0:width],
                                 slab3(kT8[par])[:, :, 128 * l:128 * (l + 1)],
                                 slab3(qT8)[:, :, 512 * s + off:512 * (s + 1)],
                                 start=True, stop=True, perf_mode=DR)
                P = pp.tile([128, 512], bf16, tag="p", name=f"p{s}_{l}{par}")
                nc.scalar.activation(P[:, 0:width], sps[:, 0:width],
                                     AF.Exp, scale=SCALE)
                if dd >= 0:
                    si = SLOT[(par, (dd, dd))]
                    nc.vector.tensor_mul(
                        P[:, 0:128], P[:, 0:128],
                        mask_sb[:, 128 * si:128 * (si + 1)])
                nc.tensor.matmul(ot[:, off:512],
                                 V[par][:, 128 * l:128 * (l + 1)],
                                 P[:, 0:width], start=first, stop=last,
                                 skip_group_check=True)
                if dps is None:
                    # denominator contribution: partition-sum of P on the
                    # idle GpSimd engine (to partition 0 of a scratch; SBUF
                    # partition offsets must be quadrant-aligned). The DVE
                    # running add into the wave accumulator is emitted a few
                    # chunks later (returned to the caller) so a Pool
                    # backlog can never block the DVE queue head.
                    if first_pool:
                        nc.gpsimd.tensor_reduce(wden[0:1, :], P[:, 0:width],
                                                axis=AX.C, op=ALU.add)
                    else:
                        dsc = dp.tile([1, 512], f32, tag="dpart",
                                      name=f"dsc{s}_{l}{par}", bufs=10)
                        nc.gpsimd.tensor_reduce(dsc[0:1, 0:width],
                                                P[:, 0:width],
                                                axis=AX.C, op=ALU.add)
                        return (dsc, off, width)
                else:
                    # Pool saturates late in a wave: late chunks put the
                    # denominator on the PE via a ones-column matmul
                    nc.tensor.matmul(dps[0:1, off:512], ones1[:],
                                     P[:, 0:width],
                                     start=dfirst, stop=dlast,
                                     skip_group_check=True)
                return None

            def finalize(s, ot, wden, dps=None):
                if dps is not None:
                    nc.vector.tensor_add(wden[0:1, :], wden[0:1, :],
                                         dps[0:1, :])
                rec = fin.tile([1, 512], f32, tag="rec", name=f"rec{s}")
                nc.vector.reciprocal_approx_fast(rec[:], wden[0:1, :])
                recB = fin.tile([128, 512], f32, tag="recB", name=f"recB{s}")
                nc.gpsimd.partition_broadcast(recB[:], rec[:])
                otn = fin.tile([128, 512], f32, tag="otn", name=f"otn{s}")
                nc.vector.tensor_mul(otn[:], ot[:], recB[:])
                otr = ps.tile([128, 512], f32, tag="s", name=f"otr{s}")
                for c in range(4):
                    nc.tensor.matmul(otr[:, 128 * c:128 * (c + 1)],
                                     otn[:, 128 * c:128 * (c + 1)], ident[:],
                                     is_transpose=True,
                                     start=(c == 0), stop=(c == 3),
                                     skip_group_check=True)
                osb = fin.tile([128, 512], f32, tag="osb", name=f"osb{s}")
                nc.scalar.copy(osb[:], otr[:])
                nc.sync.dma_start(
                    out.ap()[512 * s:512 * (s + 1), :]
                    .rearrange("(n p) h -> p n h", p=128),
                    osb[:].rearrange("p (n h) -> p n h", h=H))

            # ---- merged schedule: proj groups chased by waves.
            # Wave s's chunk stream has group-(s+1) projection cc-iters
            # spread over its first ~60% so the PE always has matmul work
            # while ACT runs exp; the group's psum->SBUF copies land at the
            # wave tail where their waits are already satisfied. The
            # other-parity group 3 is deferred into wave 3 (its k blocks
            # l>=12 come last) to give wave 3 the same PE filler. --------
            st_own0 = proj_alloc("own", 0)
            w_dma("wk")
            proj_sub_dma(st_own0, 0, xT)
            w_dma("wq")
            proj_sub_dma(st_own0, 1, xT)
            w_dma("wv")
            proj_sub_dma(st_own0, 2, xT)
            proj_sub_dma(st_own0, 3, xT)
            for cc in range(NCC):
                proj_cc(st_own0, cc)
            nc.sync.dma_start(mask_sb[:], msk.ap())
            nc.sync.dma_start(ident[:], idn.ap())
            st_oth0 = proj_dma("oth", 0, xO)
            for cc in range(NCC):
                proj_cc(st_oth0, cc)
            proj_copies(st_own0)
            proj_copies(st_oth0)

            # x DMAs for projection groups are issued one wave ahead of the
            # wave whose chunk stream carries that group's matmuls, so the
            # spread proj iters never wait on the DMA queue
            pending_fin = None
            dma_ahead = {1: [proj_dma("own", 1, xT), proj_dma("oth", 1, xO)]}
            for s in range(NSB):
                # allocate this wave's PE-den psum BEFORE any qps of a
                # later group so the pq pool's slot rotation order matches
                # actual usage order (bufs=1)
                pe_den_from = {0: 8, 1: 16, 2: 16, 3: 16}[s]
                nch = 8 * s + 8
                dps = (pq.tile([128, 512], f32, tag="qps", name=f"dps{s}")
                       if pe_den_from < nch else None)
                if s == 0:
                    dma_ahead[2] = [proj_dma("own", 2, xT),
                                    proj_dma("oth", 2, xO)]
                elif s == 1:
                    # wave 2 carries only own-g3; oth-g3 goes into wave 3
                    dma_ahead[3] = [proj_dma("own", 3, xT)]
                elif s == 2:
                    dma_ahead[4] = [proj_dma("oth", 3, xO)]
                pend = dma_ahead.pop(s + 1, [])
                chunks = [(l, par) for l in range(4 * s + 4)
                          for par in ("own", "oth")]
                # spread the pending proj cc-iters over the early chunks,
                # leaving the wave tail free so the copies' waits are met
                spread_over = 20 if s == NSB - 1 else \
                    max(1, (len(chunks) * 6) // 10)
                # own groups emit their q projection first (and its fp8
                # slab copy right after) so the next wave's qT8 dependency
                # resolves long before that wave starts
                cciters = []
                for st8 in pend:
                    if st8[5] is not None:  # qps => own group
                        cciters += [("q", st8, cc) for cc in range(NCC)]
                        cciters.append(("qcopy", st8, 0))
                for st8 in pend:
                    cciters += [("kv", st8, cc) for cc in range(NCC)]
                ncci = len(cciters)
                ot = po.tile([128, 512], f32, tag="ot", name=f"ot{s}")
                wden = dp.tile([1, 512], f32, tag="wden", name=f"wden{s}",
                               bufs=2)
                done_copies = not cciters
                pend_adds = []

                def flush_adds(upto):
                    while pend_adds and pend_adds[0][0] <= upto:
                        _, (dsc, aoff, awid) = pend_adds.pop(0)
                        nc.vector.tensor_add(wden[0:1, aoff:aoff + awid],
                                             wden[0:1, aoff:aoff + awid],
                                             dsc[0:1, 0:awid])

                for i, (l, par) in enumerate(chunks):
                    if i < pe_den_from:
                        add = chunk(s, l, par, ot, wden, i == 0)
                        if add is not None:
                            pend_adds.append((i, add))
                    else:
                        chunk(s, l, par, ot, wden, False, dps=dps,
                              dfirst=(i == pe_den_from),
                              dlast=(i == len(chunks) - 1))
                    flush_adds(i - 6)
                    if i == 10 and pending_fin is not None:
                        # previous wave's finalize chain overlaps this
                        # wave's early chunks instead of stalling the PE
                        finalize(*pending_fin)
                        pending_fin = None
                    if cciters and i < spread_over:
                        hi = ncci * (i + 1) // spread_over
                        while len(cciters) > ncci - hi:
                            kind, st8, cc = cciters.pop(0)
                            if kind == "q":
                                proj_cc_q(st8, cc)
                            elif kind == "qcopy":
                                proj_copy_q(st8)
                            else:
                                proj_cc(st8, cc)
                    if not cciters and not done_copies:
                        for st8 in pend:
                            proj_copies(st8)
                        done_copies = True
                if not done_copies:
                    for st8 in pend:
                        proj_copies(st8)
                flush_adds(len(chunks))
                pending_fin = (s, ot, wden, dps)
            finalize(*pending_fin)

    nc.finalize()
    return nc


def _build_masks(p):
    kk = np.arange(128)[:, None]   # k index (partition of S^T)
    tt = np.arange(128)[None, :]   # q index
    tri = (kk <= tt).astype(np.float32)
    ones = np.ones((128, 128), np.float32)
    zero = np.zeros((128, 128), np.float32)
    M = np.zeros((128, NSLOT * 128), np.float32)
    for (dd, c) in DIAG:
        M[:, SLOT[("own", (dd, c))] * 128:][:, :128] = \
            tri if dd == c else zero
        if p == 0:
            m = zero
        else:
            m = ones if dd == c else zero
        M[:, SLOT[("oth", (dd, c))] * 128:][:, :128] = m
    return np.ascontiguousarray(M.astype(BF16))


def _get_nc():
    if "nc" not in _cached:
        _cached["nc"] = _build_nc()
        _cached["masks"] = {p: _build_masks(p) for p in (0, 1)}
        _cached["ident32"] = np.ascontiguousarray(np.eye(128, dtype=np.float32))
    return _cached["nc"]


def _prep_in_maps(x, Wq, Wk, Wv):
    _get_nc()
    w16 = {}
    for n, w in (("Wq", Wq), ("Wk", Wk), ("Wv", Wv)):
        # SBUF layout [p, cc*128+h] = W[cc*128+p, h]
        wt = np.asarray(w).astype(BF16).reshape(NCC, 128, H)
        w16[n] = np.ascontiguousarray(
            wt.transpose(1, 0, 2).reshape(128, NCC * H))
    xTs = {}
    for b in range(B):
        xb = np.asarray(x[b])
        for p in range(2):
            sl = xb.reshape(T // 128, 128, C)[p::2].reshape(TQ, C)
            xTs[(b, p)] = np.ascontiguousarray(sl.astype(BF16).T)
    in_maps = []
    for c in range(NCORES):
        b, p = divmod(c, 2)
        in_maps.append({"xT": xTs[(b, p)], "xO": xTs[(b, 1 - p)],
                        "masks": _cached["masks"][p],
                        "ident32": _cached["ident32"], **w16})
    return in_maps


def _gather_out(results):
    out = np.empty((B, T, H), np.float32)
    for c in range(NCORES):
        b, p = divmod(c, 2)
        out[b].reshape(T // 128, 128, H)[p::2] = \
            results[c]["out"].reshape(TQ // 128, 128, H)
    return out


def kernel(x, Wq, Wk, Wv):
    from concourse.bass_utils import run_bass_kernel_spmd

    nc = _get_nc()
    in_maps = _prep_in_maps(x, Wq, Wk, Wv)
    res = run_bass_kernel_spmd(nc, in_maps, list(range(NCORES)))
    return _gather_out(res.results)
